# revision 19
# baseline (speedup 1.0000x reference)
"""2-layer GAT on 8 Trainium2 NeuronCores (Bass/Tile) — optimized.

Strategy (dst-sharded graph parallelism, heavily fused):
  - Layer-1's projection is linear, so it is FUSED into layer 0's dense
    phase: p_h = feat @ (W0_h @ W1)  [128 -> 40 per head].  The per-edge
    message matmul then directly accumulates layer-1 INPUT features:
        h2[j, c] = sum_h (1/s_jh) * sum_e alpha_raw[e,h] * p_h[src_e, c]
    and el1/er1 derive linearly from h2 (el1 = h2 @ al1).  The gathered
    row shrinks from 260 to 164 useful values.
  - All node tables are bf16 (tolerance is 2e-2; bf16 keeps ~0.4% per
    value).  Matmuls run bf16 x bf16 -> f32 PSUM (4x faster than fp32).
  - ONE physical node table per layer with TWO window VIEWS (row offset
    0 and wb_base) for the int16-indexed dma_gather; node i lives at row
    i+1, rows 0 and N+1 are zero guards shared by both windows.
  - Per-edge er comes from a TensorE trick instead of a dma_gather:
    per block, maskT = transpose(mask) via PE, then
    er_edge[e, h] = sum_j maskT[j, e] * er_tile[j, h]  (tiny matmul).
  - feat is passed pre-transposed (featT [128, N] bf16) so phase A needs
    no PE transposes: h = matmul(lhsT=featT_chunk, rhs=wfused).
  - The AllGather of the projected table is split in two so the first
    half overlaps the tail of layer-0 edge processing.

The edge structure is computed at runtime from the actual inputs and padded
to a uniform shape across cores (SPMD = one program for all 8 cores).
"""

import os
import numpy as np

import concourse.bass as bass
import concourse.bacc as bacc
import concourse.mybir as mybir
import concourse.tile as tile
from concourse.bass_utils import run_bass_kernel_spmd

F32 = mybir.dt.float32
BF16 = mybir.dt.bfloat16
I16 = mybir.dt.int16

SLOPE = 0.2
NCORES = 8
P = 128
G = 32          # max edge blocks per gather chunk
ST = 4          # dst tiles per supertile (A/B window run batching)
CH = 8          # node tiles per phase-A chunk
WROWS = 32768   # rows per index window
LAST_EXEC_NS = [None]
LAST_RES = [None]
LAST_SIM = [None]
LAST_BUILD = [None]


def _bcast_inner(apv, count):
    return bass.AP(tensor=apv.tensor, offset=apv.offset, ap=apv.ap + [[0, count]])


def _bcast_mid(apv, count):
    a = apv.ap
    return bass.AP(tensor=apv.tensor, offset=apv.offset, ap=[a[0], [0, count]] + a[1:])


def _fuse_weights(W0, al0, ar0, W1):
    """[in_dim, H0*40 fused proj | el-cols H0 | er-cols H0]  (f32)."""
    Fin = W0.shape[0]
    H, D = al0.shape
    ncls = W1.shape[1]
    W0r = W0.reshape(Fin, H, D)
    wl = np.einsum("khd,hd->kh", W0r, al0)
    wr = np.einsum("khd,hd->kh", W0r, ar0)
    W1r = W1.reshape(H, D, ncls)
    # p_h = feat @ (W0_h @ W1_h)
    fused = np.einsum("khd,hdc->khc", W0r, W1r).reshape(Fin, H * ncls)
    return np.ascontiguousarray(
        np.concatenate([fused, wl, wr], axis=1), dtype=np.float32)


def _wrap16(idx):
    """int16 idx list (len multiple of 128) -> dma_gather SBUF layout
    [128, len/16]: idx j at [j % 16, j // 16], replicated across 8 groups."""
    w = idx.reshape(-1, 16).T.astype(np.int16)
    return np.ascontiguousarray(np.tile(w, (8, 1)))


def _prep_edges(src, dst, n_nodes, ncores, wrows):
    """Sort edges by dst, shard by dst range, split per (tile, window) with
    flex assignment (srcs valid in both windows fill window A to a multiple
    of 128 first), pad blocks, and order blocks per supertile: all window-A
    blocks of ST consecutive tiles, then all window-B blocks."""
    from types import SimpleNamespace
    plan = SimpleNamespace()
    npc = n_nodes // ncores
    tpc = (npc + P - 1) // P
    plan.npc, plan.tpc = npc, tpc
    wa_max = wrows - 2               # node i valid in A iff i+1 <= wrows-1
    wb_base = n_nodes + 2 - wrows    # node i valid in B iff i >= wb_base - 1
    plan.wb_base = wb_base

    order = np.argsort(dst, kind="stable")
    ss = src[order].astype(np.int64)
    ds = dst[order].astype(np.int64)
    core = ds // npc
    loc = ds % npc
    tileid = loc // P

    # category: 0 = A-only, 1 = flex (valid both), 2 = B-only
    cat = np.where(ss > wa_max, 2, np.where(ss >= wb_base - 1, 1, 0))

    # per (core, tile): counts
    nA = np.zeros((ncores, tpc), np.int64)
    nF = np.zeros((ncores, tpc), np.int64)
    nB = np.zeros((ncores, tpc), np.int64)
    np.add.at(nA, (core, tileid), cat == 0)
    np.add.at(nF, (core, tileid), cat == 1)
    np.add.at(nB, (core, tileid), cat == 2)

    # flex to A: fill A to a multiple of 128 (so only B's last block pads),
    # remainder of flex goes to B.
    want = (P - (nA % P)) % P
    fa = np.minimum(nF, want)
    # if flex can't fill A to x128, put ALL flex in A only if B is empty
    # (avoids a second padded block); otherwise dump remainder into B.
    fb = nF - fa
    cntA = nA + fa
    cntB = nB + fb
    # per-tile block counts = max over cores (SPMD uniform shape)
    nblkA = (cntA + P - 1) // P
    nblkB = (cntB + P - 1) // P
    bcntA = nblkA.max(axis=0)
    bcntB = nblkB.max(axis=0)
    if bcntA.sum() + bcntB.sum() == 0:
        bcntA[0] = 1

    sts = [list(range(s, min(s + ST, tpc))) for s in range(0, tpc, ST)]
    plan.sts = sts
    plan.order_blocks = []
    for tiles in sts:
        for w in (0, 1):
            bc = bcntA if w == 0 else bcntB
            for t in tiles:
                plan.order_blocks += [(t, w)] * int(bc[t])
    plan.totblk = len(plan.order_blocks)
    plan.nedge = plan.totblk * P

    slot = {}
    pos = 0
    for (t, w) in plan.order_blocks:
        if (t, w) not in slot:
            slot[(t, w)] = pos
        pos += P

    srcw = np.zeros((ncores, plan.nedge), np.int64)
    dstloc = np.full((ncores, plan.nedge), 999.0, np.float32)
    for bi, (t, w) in enumerate(plan.order_blocks):
        if w == 1:
            srcw[:, bi * P:(bi + 1) * P] = wrows - 1  # B guard row (zero)
        # A pad stays 0 (guard row zero)

    # assign each edge its (tile, window) group; order within group stable
    towin = np.zeros(len(ss), np.int64)
    towin[cat == 2] = 1
    # flex edges: first `fa[core,tile]` of each tile's flex run go to A
    flexmask = cat == 1
    if flexmask.any():
        # rank of each flex edge within its (core, tile) group
        key = core * tpc + tileid
        fkey = key[flexmask]
        forder = np.argsort(fkey, kind="stable")
        ranks = np.empty(len(fkey), np.int64)
        kk = fkey[forder]
        starts = np.r_[0, np.flatnonzero(np.diff(kk)) + 1]
        grp = np.zeros(len(kk), np.int64)
        grp[starts[1:]] = 1
        grp = np.cumsum(grp)
        within = np.arange(len(kk)) - starts[grp]
        ranks[forder] = within
        fcore = core[flexmask][forder]
        ftile = tileid[flexmask][forder]
        toB = within >= fa[fcore, ftile]
        tw = np.zeros(len(fkey), np.int64)
        tw[forder] = toB
        towin[flexmask] = tw

    gkey = (core * tpc + tileid) * 2 + towin
    g_order = np.argsort(gkey, kind="stable")
    ss2, loc2, gkey2 = ss[g_order], loc[g_order], gkey[g_order]
    gstart = np.zeros(ncores * tpc * 2 + 1, np.int64)
    np.add.at(gstart[1:], gkey2, 1)
    gstart = np.cumsum(gstart)
    for c in range(ncores):
        for t in range(tpc):
            for w in (0, 1):
                k = (c * tpc + t) * 2 + w
                e0, e1 = int(gstart[k]), int(gstart[k + 1])
                cnt = e1 - e0
                if cnt == 0:
                    continue
                off = slot[(t, w)]
                srcs = ss2[e0:e1]
                srcw[c, off:off + cnt] = (
                    srcs + 1 if w == 0 else srcs + 1 - wb_base)
                dstloc[c, off:off + cnt] = (loc2[e0:e1] % P).astype(np.float32)

    plan.srcw, plan.dstlocv = srcw, dstloc
    return plan


def _chunks_of_blocks(order_blocks, g):
    """Maximal runs of <= g blocks within a single window."""
    chunks = []
    cur = None
    for bi, (t, w) in enumerate(order_blocks):
        if cur is None or cur[0] != w or bi - cur[1] >= g:
            if cur is not None:
                chunks.append(cur)
            cur = [w, bi, bi + 1]
        else:
            cur[2] = bi + 1
        if cur[2] - cur[1] >= g:
            chunks.append(cur)
            cur = None
    if cur is not None:
        chunks.append(cur)
    return chunks


def build_and_run(feat, src, dst, W0, al0, ar0, W1, al1, ar1, trace=False,
                  simulate=False):
    n_nodes = feat.shape[0]
    npc = n_nodes // NCORES
    nh0 = al0.shape[0]          # 4
    ncls = W1.shape[1]          # 40 (H1 == 1)
    nh1 = al1.shape[0]          # 1
    d0 = nh0 * ncls             # 160: fused projected width
    row0 = 256                  # bf16 elems per tab0 row (512B, x256B ok)
    row1 = 128                  # bf16 elems per tab1 row (256B)
    in_dim = feat.shape[1]
    wrows = min(WROWS, n_nodes + 2)
    wb_base = n_nodes + 2 - wrows
    EL0 = d0                    # el cols [160,164), er cols [164,168)
    ER0 = d0 + nh0
    ROW0W = d0 + 2 * nh0        # 168 useful cols in tab0
    ROW1W = ncls + 2 * nh1 + 1  # 43 cols in tab1 (h2 | el1 | er1 | er1 dup)

    wf0 = _fuse_weights(W0, al0, ar0, W1)          # [in_dim, 168]
    featT = np.ascontiguousarray(feat.T, dtype=np.float32)

    plan = _prep_edges(src, dst, n_nodes, NCORES, wrows)
    totblk = plan.totblk
    tpc = plan.tpc
    npc_pad = tpc * P
    nseg = npc_pad // P
    chunks = _chunks_of_blocks(plan.order_blocks, G)

    # per-tile first/last block ids
    first_blk, last_blk = {}, {}
    for bi, (t, w) in enumerate(plan.order_blocks):
        if t not in first_blk:
            first_blk[t] = bi
        last_blk[t] = bi

    # AllGather split points at supertile boundaries
    nst = len(plan.sts)
    nag = int(os.environ.get("GAT_AGPARTS", "2"))
    if nst >= nag and nag > 1:
        marks = sorted({plan.sts[(nst * i) // nag - 1][-1]
                        for i in range(1, nag)})
        marks = [t for t in marks if t < tpc - 1]
    else:
        marks = []
    ag_tiles = marks + [tpc - 1]

    # erloc build index lists (local node -> window row; invalid -> guard)
    gidx = np.arange(npc_pad, dtype=np.int64)
    bia = np.zeros((NCORES, npc_pad), np.int64)
    bib = np.full((NCORES, npc_pad), wrows - 1, np.int64)
    for c in range(NCORES):
        g = c * npc + gidx
        valid = gidx < npc
        a_ok = valid & (g + 1 <= wrows - 1)
        b_ok = valid & ~a_ok
        bia[c, :] = np.where(a_ok, g + 1, 0)
        bib[c, :] = np.where(b_ok, g + 1 - wb_base, wrows - 1)

    iota = np.broadcast_to(np.arange(P, dtype=np.float32), (P, P)).copy()
    ident = np.eye(P, dtype=np.float32)
    alr = np.stack([np.tile(al1.reshape(-1)[:ncls], (P, 1)),
                    np.tile(ar1.reshape(-1)[:ncls], (P, 1))], axis=1)  # [P,2,ncls]

    nc = bacc.Bacc(None, target_bir_lowering=False, num_devices=NCORES)
    featT_t = nc.declare_dram_parameter("featT", [in_dim, n_nodes], BF16, False)
    wf0_t = nc.declare_dram_parameter("wf0", [in_dim, ROW0W], BF16, False)
    iota_t = nc.declare_dram_parameter("iota", [P, P], BF16, False)
    ident_t = nc.declare_dram_parameter("ident", [P, P], BF16, False)
    alr_t = nc.declare_dram_parameter("alr", [P, 2, ncls], BF16, False)
    src16_t = nc.declare_dram_parameter("src16", [P, totblk * 8], I16, False)
    dstloc_t = nc.declare_dram_parameter("dstloc", [P, totblk], BF16, False)
    bia_t = nc.declare_dram_parameter("bia16", [P, npc_pad // 16], I16, False)
    bib_t = nc.declare_dram_parameter("bib16", [P, npc_pad // 16], I16, False)
    out_t = nc.declare_dram_parameter("out", [npc, ncls], F32, True)

    tab0 = nc.dram_tensor("tab0", [n_nodes + 2, row0], BF16)
    tab1 = nc.dram_tensor("tab1", [n_nodes + 2, row1], BF16)
    h2slice = nc.dram_tensor("h2slice", [npc, ROW1W], BF16)

    tab0A = tab0.ap()[0:wrows]
    tab0B = tab0.ap()[wb_base:wb_base + wrows]
    tab1A = tab1.ap()[0:wrows]
    tab1B = tab1.ap()[wb_base:wb_base + wrows]

    nt_full = n_nodes // P
    rem = n_nodes - nt_full * P

    with tile.TileContext(nc) as tc:
        with tc.tile_pool(name="singles", bufs=1) as singles:
            iota_sb = singles.tile([P, P], BF16)
            nc.sync.dma_start(out=iota_sb[:], in_=iota_t.ap())
            ident_sb = singles.tile([P, P], BF16)
            nc.sync.dma_start(out=ident_sb[:], in_=ident_t.ap())
            wf0_sb = singles.tile([P, ROW0W], BF16)
            nc.sync.dma_start(out=wf0_sb[:], in_=wf0_t.ap())
            alr_sb = singles.tile([P, 2, ncls], BF16)
            nc.sync.dma_start(out=alr_sb[:], in_=alr_t.ap())
            src16_sb = singles.tile([P, totblk * 8], I16)
            nc.sync.dma_start(out=src16_sb[:], in_=src16_t.ap())
            dstloc_sb = singles.tile([P, totblk], BF16)
            nc.sync.dma_start(out=dstloc_sb[:], in_=dstloc_t.ap())
            bia_sb = singles.tile([P, npc_pad // 16], I16)
            nc.sync.dma_start(out=bia_sb[:], in_=bia_t.ap())
            bib_sb = singles.tile([P, npc_pad // 16], I16)
            nc.sync.dma_start(out=bib_sb[:], in_=bib_t.ap())
            zrow = singles.tile([P, row0], BF16)
            nc.vector.memset(zrow[:], 0.0)
            # zero guard rows (0 and n_nodes+1) of both tables
            nc.sync.dma_start(out=tab0.ap()[0:1], in_=zrow[:1, :row0])
            nc.sync.dma_start(out=tab0.ap()[n_nodes + 1:n_nodes + 2],
                              in_=zrow[:1, :row0])
            nc.sync.dma_start(out=tab1.ap()[0:1], in_=zrow[:1, :row1])
            nc.sync.dma_start(out=tab1.ap()[n_nodes + 1:n_nodes + 2],
                              in_=zrow[:1, :row1])

            # ---- Phase A: replicated fused dense layer -> tab0 ----
            with (tc.tile_pool(name="pa_ft", bufs=3) as pa_ft,
                  tc.tile_pool(name="pa_hs", bufs=2) as pa_hs,
                  tc.tile_pool(name="pa_ps", bufs=2, space="PSUM") as pa_ps):
                base = 0
                while base < n_nodes:
                    ch_full = min(CH, (n_nodes - base) // P)
                    partial = ch_full == 0
                    ntl = max(ch_full, 1)
                    rows = rem if partial else ntl * P
                    ft = pa_ft.tile([P, CH * P], BF16, tag="ft", name="ft")
                    nc.sync.dma_start(out=ft[:, 0:rows],
                                      in_=featT_t.ap()[:, base:base + rows])
                    hstage = pa_hs.tile([P, CH, ROW0W], BF16, tag="hs",
                                        name="hs")
                    for i in range(ntl):
                        cols = rows - i * P if partial else P
                        hps = pa_ps.tile([P, ROW0W], F32, name="hps")
                        nc.tensor.matmul(hps[:cols, :],
                                         lhsT=ft[:, i * P:i * P + cols],
                                         rhs=wf0_sb[:], start=True, stop=True)
                        nc.scalar.copy(out=hstage[:cols, i, :],
                                       in_=hps[:cols, :])
                    if partial:
                        nc.sync.dma_start(
                            out=tab0.ap()[base + 1:base + 1 + rows, 0:ROW0W],
                            in_=hstage[:rows, 0, :])
                    else:
                        nc.sync.dma_start(
                            out=tab0.ap()[base + 1:base + 1 + rows,
                                          0:ROW0W].rearrange(
                                              "(i p) d -> p i d", p=P),
                            in_=hstage[:, :ntl, :])
                    base += rows

            # ---- shared pools for edge phases ----
            with (tc.tile_pool(name="hg", bufs=2) as hg_pool,
                  tc.tile_pool(name="ms", bufs=2) as ms_pool,
                  tc.tile_pool(name="mk", bufs=2) as mask_pool,
                  tc.tile_pool(name="mts", bufs=3) as mts_pool,
                  tc.tile_pool(name="sm", bufs=3) as small_pool,
                  tc.tile_pool(name="fin", bufs=2) as fin_pool,
                  tc.tile_pool(name="erl", bufs=1) as erl_pool,
                  tc.tile_pool(name="ps_acc", bufs=4, space="PSUM") as psum_acc,
                  tc.tile_pool(name="ps_tp", bufs=2, space="PSUM") as psum_tp,
                  tc.tile_pool(name="ps_er", bufs=2, space="PSUM") as psum_er):

                def build_erloc(tabA_ap, tabB_ap, width, col0, tag):
                    """Gather this core's local-node er columns into SBUF:
                    out[p, s, :] = table row (s*128+p) cols [col0, col0+128)."""
                    EA = erl_pool.tile([P, nseg, P], BF16, tag=tag + "a",
                                       name=tag + "a")
                    nc.gpsimd.dma_gather(
                        out_ap=EA[:], in_ap=tabA_ap[:, col0:col0 + P],
                        idxs_ap=bia_sb[:], num_idxs=npc_pad,
                        num_idxs_reg=npc_pad, elem_size=P, elem_step=width,
                        single_packet=False)
                    EB = erl_pool.tile([P, nseg, P], BF16, tag=tag + "b",
                                       name=tag + "b")
                    nc.gpsimd.dma_gather(
                        out_ap=EB[:], in_ap=tabB_ap[:, col0:col0 + P],
                        idxs_ap=bib_sb[:], num_idxs=npc_pad,
                        num_idxs_reg=npc_pad, elem_size=P, elem_step=width,
                        single_packet=False)
                    nc.vector.tensor_add(EA[:], EA[:], EB[:])
                    return EA

                def edge_phase(tabA_ap, tabB_ap, er_sb, er_col, nheads, hdim,
                               gw, finalize, post_tile):
                    """gw = gathered row elems; row = [p(nheads*hdim) | el
                    (nheads) | ...]; er_tile(t) = er_sb[:, t, er_col:+nheads]"""
                    d = nheads * hdim
                    msw = d + nheads
                    erw = max(nheads, 2)   # 1-wide matmuls crash the PE
                    acc_by_tile = {}
                    for w, b0, b1 in chunks:
                        nb = b1 - b0
                        nidx = nb * P
                        HG = hg_pool.tile([P, G, gw], BF16, tag=f"hg{gw}",
                                          name="hg")
                        nc.gpsimd.dma_gather(
                            out_ap=HG[:, :nb, :],
                            in_ap=(tabA_ap if w == 0 else tabB_ap),
                            idxs_ap=src16_sb[:, b0 * 8:b1 * 8], num_idxs=nidx,
                            num_idxs_reg=nidx, elem_size=gw, elem_step=gw,
                            single_packet=False)
                        MASK = mask_pool.tile([P, G, P], BF16, tag="mask",
                                              name="mask")
                        nc.vector.tensor_tensor(
                            out=MASK[:, :nb, :],
                            in0=_bcast_inner(dstloc_sb[:, b0:b1], P),
                            in1=_bcast_mid(iota_sb[:], nb),
                            op=mybir.AluOpType.is_equal)
                        # per-block: maskT via PE, tiny er matmul into ER_PS
                        skip_er = os.environ.get("GAT_NOER", "0") == "1"
                        ER_PS = psum_er.tile([P, G * erw], F32, name="erps")
                        if not skip_er:
                            TB = 4   # blocks per batched maskT PSUM->SBUF copy
                            for kb in range(0, nb, TB):
                                kn = min(TB, nb - kb)
                                mtp = psum_tp.tile([P, TB, P], BF16,
                                                   name="mtp")
                                for k in range(kb, kb + kn):
                                    nc.tensor.transpose(mtp[:, k - kb, :],
                                                        MASK[:, k, :],
                                                        ident_sb[:])
                                mts = mts_pool.tile([P, TB, P], BF16,
                                                    tag="mts", name="mts")
                                # alternate ACT/DVE to balance engines
                                if (kb // TB) % 2 == 0:
                                    nc.scalar.copy(out=mts[:, :kn, :],
                                                   in_=mtp[:, :kn, :])
                                else:
                                    nc.vector.tensor_copy(mts[:, :kn, :],
                                                          mtp[:, :kn, :])
                                for k in range(kb, kb + kn):
                                    t, _ = plan.order_blocks[b0 + k]
                                    nc.tensor.matmul(
                                        ER_PS[:, k * erw:k * erw + erw],
                                        lhsT=mts[:, k - kb, :],
                                        rhs=er_sb[:, t, er_col:er_col + erw],
                                        start=True, stop=True)
                        # x = el + er ; expe = exp(max(x, slope*x))
                        E4 = small_pool.tile([P, G, nheads], F32, tag="e4",
                                             name="e4")
                        if skip_er:
                            nc.vector.tensor_copy(E4[:, :nb, :],
                                                  HG[:, :nb, d:d + nheads])
                        else:
                            nc.vector.tensor_add(
                                E4[:, :nb, :], HG[:, :nb, d:d + nheads],
                                ER_PS[:].rearrange("p (b h) -> p b h",
                                                   h=erw)[:, :nb, 0:nheads])
                        ESC = small_pool.tile([P, G, nheads], F32, tag="esc",
                                              name="esc")
                        nc.vector.tensor_scalar_mul(ESC[:, :nb, :],
                                                    E4[:, :nb, :], SLOPE)
                        nc.vector.tensor_tensor(out=E4[:, :nb, :],
                                                in0=E4[:, :nb, :],
                                                in1=ESC[:, :nb, :],
                                                op=mybir.AluOpType.max)
                        nc.scalar.activation(
                            out=E4[:, :nb, :], in_=E4[:, :nb, :],
                            func=mybir.ActivationFunctionType.Exp)
                        # MS = [expe-scaled p | expe]
                        MS = ms_pool.tile([P, G, msw], BF16, tag=f"ms{msw}",
                                          name="ms")
                        for h in range(nheads):
                            nc.vector.tensor_tensor(
                                out=MS[:, :nb, h * hdim:(h + 1) * hdim],
                                in0=HG[:, :nb, h * hdim:(h + 1) * hdim],
                                in1=_bcast_inner(E4[:, :nb, h:h + 1], hdim),
                                op=mybir.AluOpType.mult)
                        nc.scalar.copy(out=MS[:, :nb, d:d + nheads],
                                       in_=E4[:, :nb, :])
                        for k in range(nb):
                            bi = b0 + k
                            t, _ = plan.order_blocks[bi]
                            if bi == first_blk[t]:
                                acc_by_tile[t] = psum_acc.tile(
                                    [P, msw], F32, tag="acc", name="acc")
                            acc = acc_by_tile[t]
                            nc.tensor.matmul(acc[:], lhsT=MASK[:, k, :],
                                             rhs=MS[:, k, :],
                                             start=(bi == first_blk[t]),
                                             stop=(bi == last_blk[t]))
                            if bi == last_blk[t]:
                                finalize(t, acc)
                                del acc_by_tile[t]
                                if post_tile is not None:
                                    post_tile(t)

                # ---- Layer 0 ----
                phases = os.environ.get("GAT_PHASES", "full")
                er0_sb = (build_erloc(tab0A, tab0B, row0, P, "er0")
                          if phases != "a" else None)
                er0_col = ER0 - P   # er cols within gathered [128, 256) slice

                def finalize0(t, acc):
                    rows = min(P, npc - t * P)
                    S = small_pool.tile([P, nh0], F32, tag="s0", name="s0")
                    nc.vector.tensor_scalar_max(S[:], acc[:, d0:d0 + nh0],
                                                1e-30)
                    RC = small_pool.tile([P, nh0], F32, tag="rc0", name="rc0")
                    nc.vector.reciprocal(RC[:], S[:])
                    H2 = fin_pool.tile([P, ncls], F32, tag="h2f", name="h2f")
                    TMP = fin_pool.tile([P, ncls], F32, tag="tmpf",
                                        name="tmpf")
                    nc.vector.tensor_scalar_mul(H2[:], acc[:, 0:ncls],
                                                RC[:, 0:1])
                    for h in range(1, nh0):
                        nc.vector.tensor_scalar_mul(
                            TMP[:], acc[:, h * ncls:(h + 1) * ncls],
                            RC[:, h:h + 1])
                        nc.vector.tensor_add(H2[:], H2[:], TMP[:])
                    # el1 = H2 @ al1, er1 = H2 @ ar1 (via DVE mult+reduce)
                    EL = fin_pool.tile([P, 2, ncls], F32, tag="elf",
                                       name="elf")
                    nc.vector.tensor_tensor(
                        out=EL[:], in0=_bcast_mid(H2[:], 2), in1=alr_sb[:],
                        op=mybir.AluOpType.mult)
                    ELR = small_pool.tile([P, 2], F32, tag="elr", name="elr")
                    nc.vector.tensor_reduce(ELR[:], EL[:],
                                            axis=mybir.AxisListType.X,
                                            op=mybir.AluOpType.add)
                    h2sb = fin_pool.tile([P, ROW1W], BF16, tag="h2sb",
                                         name="h2sb")
                    nc.vector.tensor_copy(h2sb[:, 0:ncls], H2[:])
                    nc.vector.tensor_copy(h2sb[:, ncls:ncls + 2], ELR[:])
                    # duplicate er1 so the L1 er matmul can be 2 wide
                    nc.vector.tensor_copy(h2sb[:, ncls + 2:ncls + 3],
                                          ELR[:, 1:2])
                    nc.sync.dma_start(out=h2slice.ap()[t * P:t * P + rows],
                                      in_=h2sb[:rows, :])

                ag_parts = []
                prev = 0
                for t in ag_tiles:
                    r1 = min((t + 1) * P, npc)
                    if r1 > prev:
                        ag_parts.append((t, prev, r1))
                        prev = r1
                ag_by_tile = {t: (r0, r1) for (t, r0, r1) in ag_parts}
                h2parts = {
                    t: nc.dram_tensor(f"h2full{i}",
                                      [NCORES, r1 - r0, ROW1W], BF16,
                                      addr_space="Shared")
                    for i, (t, r0, r1) in enumerate(ag_parts)}

                def post_tile0(t):
                    if t not in ag_by_tile:
                        return
                    r0, r1 = ag_by_tile[t]
                    part = h2parts[t]
                    nc.gpsimd.collective_compute(
                        "AllGather", mybir.AluOpType.bypass,
                        replica_groups=[list(range(NCORES))],
                        ins=[h2slice.ap()[r0:r1]],
                        outs=[part.ap()])
                    # repack rows [c*npc + r0 + 1, c*npc + r1 + 1) per core
                    nrow = r1 - r0
                    for c in range(NCORES):
                        o = c * npc + r0 + 1
                        nc.sync.dma_start(
                            out=tab1.ap()[o:o + nrow, 0:ROW1W],
                            in_=part.ap()[c])

                if phases not in ("a", "ae"):
                    edge_phase(tab0A, tab0B, er0_sb, er0_col, nh0, ncls,
                               row0, finalize0,
                               post_tile0 if phases != "l0" else None)

                # ---- Layer 1 ----
                run_l1 = phases == "full"
                er1_sb = (build_erloc(tab1A, tab1B, row1, 0, "er1")
                          if phases in ("full", "ag") else None)
                er1_col = ncls + nh1   # er1 at col 41

                def finalize1(t, acc):
                    rows = min(P, npc - t * P)
                    S = small_pool.tile([P, nh1], F32, tag="s1", name="s1")
                    nc.vector.tensor_scalar_max(S[:], acc[:, ncls:ncls + nh1],
                                                1e-30)
                    RC = small_pool.tile([P, nh1], F32, tag="rc1", name="rc1")
                    nc.vector.reciprocal(RC[:], S[:])
                    OUT = fin_pool.tile([P, ncls], F32, tag="outt",
                                        name="outt")
                    nc.vector.tensor_scalar_mul(OUT[:], acc[:, 0:ncls],
                                                RC[:, 0:1])
                    nc.sync.dma_start(out=out_t.ap()[t * P:t * P + rows],
                                      in_=OUT[:rows, :])

                if run_l1:
                    edge_phase(tab1A, tab1B, er1_sb, er1_col, nh1, ncls,
                               row1, finalize1, None)
                else:
                    zout = fin_pool.tile([P, ncls], F32, tag="outt",
                                         name="zout")
                    nc.vector.memset(zout[:], 0.0)
                    for t in range(tpc):
                        rows = min(P, npc - t * P)
                        nc.sync.dma_start(
                            out=out_t.ap()[t * P:t * P + rows],
                            in_=zout[:rows, :])

    nc.compile()

    in_maps = []
    import ml_dtypes
    bf = ml_dtypes.bfloat16
    featT_bf = featT.astype(bf)
    wf0_bf = wf0.astype(bf)
    iota_bf = iota.astype(bf)
    ident_bf = ident.astype(bf)
    alr_bf = alr.astype(bf)
    for c in range(NCORES):
        in_maps.append({
            "featT": featT_bf,
            "wf0": wf0_bf,
            "iota": iota_bf,
            "ident": ident_bf,
            "alr": alr_bf,
            "src16": _wrap16(plan.srcw[c]),
            "dstloc": np.ascontiguousarray(
                plan.dstlocv[c].reshape(totblk, P).T).astype(bf),
            "bia16": np.ascontiguousarray(
                np.tile(bia[c].reshape(-1, 16).T.astype(np.int16), (8, 1))),
            "bib16": np.ascontiguousarray(
                np.tile(bib[c].reshape(-1, 16).T.astype(np.int16), (8, 1))),
        })
    LAST_BUILD[0] = (nc, in_maps)
    if simulate:
        from concourse import bass_interp
        sim = bass_interp.MultiCoreSim(nc, NCORES, ignore_data_errors=True)
        for c in range(NCORES):
            for k, v in in_maps[c].items():
                sim.cores[c].tensor(k)[:] = v
        sim.simulate()
        LAST_SIM[0] = sim
        out = np.concatenate(
            [np.array(sim.cores[c].tensor("out")) for c in range(NCORES)],
            axis=0)
        return out
    res = run_bass_kernel_spmd(nc, in_maps, list(range(NCORES)), trace=trace)
    LAST_RES[0] = res
    LAST_EXEC_NS[0] = res.exec_time_ns
    out = np.concatenate([res.results[c]["out"] for c in range(NCORES)],
                         axis=0)
    return out


def kernel(feat, src, dst, W0, al0, ar0, W1, al1, ar1):
    trace = os.environ.get("GAT_TRACE", "0") == "1"
    out = build_and_run(np.asarray(feat), np.asarray(src), np.asarray(dst),
                        np.asarray(W0), np.asarray(al0), np.asarray(ar0),
                        np.asarray(W1), np.asarray(al1), np.asarray(ar1),
                        trace=trace)
    return out.astype(np.float32)
